# revision 6
# baseline (speedup 1.0000x reference)
"""Trainium2 Bass kernel for the AMTCL loss (nn_AMTCL_66520453480770).

Math: the reference builds a [B,B] pairwise distance matrix dist[i,j] between
inputs[i] and centers[targets[j]] (weights 2**centers_weights[targets[j]]).
Since dist[i,j] depends on j only through c = targets[j], the whole problem
collapses to the [B,C] matrix
    dc[i,c] = sqrt(sum_d w2[c,d] * (centers[c,d] - inputs[i,d])**2)
with
    dist_ap[i] = dc[i, t_i]                      (all same-class j are equal)
    dist_an[i] = min_{c present, c != t_i} dc[i,c]
    cc[i]      = centers_dist[t_i],  centers_dist[c] = min_{j!=c} cd[c,j]
    loss       = mean(dist_ap + relu(cc - dist_an))
This is exactly equal to the reference (40x less compute than the B^2 form).

dc2 is computed as ONE GEMM with contraction K = 2D+1:
    dc2[i,c] = sum_d xsq[i,d]*w2[c,d] + sum_d x[i,d]*(-2*w2[c,d]*c[c,d]) + a[c]
stationary = [xsqT; xT; 1] (X side), moving = [w2T; -2*w2T*cT; a_row].

Sharding: data-parallel over the 4096 anchor rows -> 8 cores x 512 rows.
centers/centers_weights replicated. Each core emits a partial loss sum [1,1];
the host sums the 8 scalars and divides by B.
"""

import numpy as np

import concourse.bass as bass
import concourse.bacc as bacc
import concourse.mybir as mybir
import concourse.tile as tile
from concourse.bass_utils import run_bass_kernel_spmd

B, C, D = 4096, 100, 384
NCORES = 8
ROWS = B // NCORES          # 512 anchor rows per core
MCH = ROWS // 128           # 4 partition chunks of anchor rows
KD = D // 128               # 3 partition chunks of the feature dim
BIG = float(2 ** 20)        # self-class exclusion offset (exact power of 2)
PEN = float(2 ** 40)        # absent-class / diagonal penalty (pre-sqrt)
LN2 = float(np.log(2.0))
F32 = mybir.dt.float32


def build_nc() -> bass.Bass:
    nc = bacc.Bacc(
        "TRN2", target_bir_lowering=False, debug=False, num_devices=NCORES
    )

    xT = nc.declare_dram_parameter("xT", [D, ROWS], F32, isOutput=False)
    cT = nc.declare_dram_parameter("cT", [D, C], F32, isOutput=False)
    cwT = nc.declare_dram_parameter("cwT", [D, C], F32, isOutput=False)
    tgt = nc.declare_dram_parameter("tgt", [MCH, 128], F32, isOutput=False)
    pen = nc.declare_dram_parameter("pen", [1, C], F32, isOutput=False)
    iota = nc.declare_dram_parameter("iota", [128, C], F32, isOutput=False)
    ident = nc.declare_dram_parameter("ident", [C, C], F32, isOutput=False)
    ones = nc.declare_dram_parameter("ones", [128, 128], F32, isOutput=False)
    out = nc.declare_dram_parameter("out", [1, 1], F32, isOutput=True)

    with tile.TileContext(nc) as tc:
        with (
            tc.tile_pool(name="const", bufs=1) as cp,
            tc.tile_pool(name="wts", bufs=1) as wp,
            tc.tile_pool(name="work", bufs=2) as kp,
            tc.tile_pool(name="ps1", bufs=1, space="PSUM") as pp1,
            tc.tile_pool(name="ps2", bufs=2, space="PSUM") as pp2,
        ):
            # ---- constants / small inputs ----
            ones_sb = cp.tile([128, 128], F32, tag="ones")
            nc.sync.dma_start(ones_sb[:], ones[:])
            iota_sb = cp.tile([128, C], F32, tag="iota")
            nc.sync.dma_start(iota_sb[:], iota[:])
            ident_sb = cp.tile([C, C], F32, tag="ident")
            nc.sync.dma_start(ident_sb[:], ident[:])
            pen_sb = cp.tile([1, C], F32, tag="pen")
            nc.sync.dma_start(pen_sb[:], pen[:])
            t_sb = cp.tile([128, MCH], F32, tag="tsb")
            nc.sync.dma_start(t_sb[:], tgt.rearrange("c p -> p c"))

            # ---- center-side prep: w2T, -2*w2T*cT, csqT, a_row ----
            cT_sb, w2T_sb, csqT_sb, m2T_sb = [], [], [], []
            psum_arow = pp1.tile([1, C], F32, tag="arow")
            for k in range(KD):
                csb = wp.tile([128, C], F32, tag=f"cT{k}")
                nc.sync.dma_start(csb[:], cT[k * 128 : (k + 1) * 128, :])
                cT_sb.append(csb)
                cwsb = kp.tile([128, C], F32, tag="cwT")
                nc.sync.dma_start(cwsb[:], cwT[k * 128 : (k + 1) * 128, :])
                w2 = wp.tile([128, C], F32, tag=f"w2T{k}")
                # 2**x = exp(x * ln2)
                nc.scalar.activation(
                    w2[:], cwsb[:], mybir.ActivationFunctionType.Exp, scale=LN2
                )
                w2T_sb.append(w2)
                csq = wp.tile([128, C], F32, tag=f"csqT{k}")
                nc.scalar.square(csq[:], csb[:])
                csqT_sb.append(csq)
                w2c = kp.tile([128, C], F32, tag="w2c")
                nc.vector.tensor_tensor(
                    w2c[:], w2[:], csb[:], op=mybir.AluOpType.mult
                )
                m2 = wp.tile([128, C], F32, tag=f"m2T{k}")
                nc.scalar.mul(m2[:], w2c[:], -2.0)
                m2T_sb.append(m2)
                wsq = kp.tile([128, C], F32, tag="wsq")
                nc.vector.tensor_tensor(
                    wsq[:], w2[:], csq[:], op=mybir.AluOpType.mult
                )
                nc.tensor.matmul(
                    psum_arow[:], ones_sb[:, 0:1], wsq[:],
                    start=(k == 0), stop=(k == KD - 1),
                )
            arow_sb = wp.tile([1, C], F32, tag="arow_sb")
            # a_row + penalty for globally-absent classes (normally all zeros)
            nc.vector.tensor_tensor(
                arow_sb[:], psum_arow[:], pen_sb[:], op=mybir.AluOpType.add
            )

            # ---- centers_dist: cd2[i,j] = a[i] - 2*(w2c)[i]·c[j] + w2[i]·csq[j]
            psum_cd2 = pp1.tile([C, C], F32, tag="cd2")
            for k in range(KD):
                nc.tensor.matmul(
                    psum_cd2[:], m2T_sb[k][:], cT_sb[k][:],
                    start=(k == 0), stop=False,
                )
                nc.tensor.matmul(
                    psum_cd2[:], w2T_sb[k][:], csqT_sb[k][:],
                    start=False, stop=False,
                )
            nc.tensor.matmul(
                psum_cd2[:], arow_sb[:], ones_sb[0:1, 0:C],
                start=False, stop=True,
            )
            cd2m_sb = wp.tile([C, C], F32, tag="cd2m")
            nc.vector.tensor_scalar(
                cd2m_sb[:], psum_cd2[:], 0.0, None, op0=mybir.AluOpType.max
            )
            # exclude the diagonal pre-sqrt: iota(f - p) != 0 keeps, else PEN
            cd2x_sb = wp.tile([C, C], F32, tag="cd2x")
            nc.gpsimd.affine_select(
                cd2x_sb[:], cd2m_sb[:], pattern=[[1, C]],
                compare_op=mybir.AluOpType.not_equal, fill=PEN,
                base=0, channel_multiplier=-1,
            )
            cd_sb = wp.tile([C, C], F32, tag="cd")
            nc.scalar.sqrt(cd_sb[:], cd2x_sb[:])
            cdmin_sb = wp.tile([C, 1], F32, tag="cdmin")
            nc.vector.tensor_reduce(
                cdmin_sb[:], cd_sb[:], axis=mybir.AxisListType.X,
                op=mybir.AluOpType.min,
            )
            # transpose [C,1] -> [1,C] via identity matmul, then bcast to 128 rows
            psum_cdrow = pp1.tile([1, C], F32, tag="cdrow")
            nc.tensor.matmul(psum_cdrow[:], cdmin_sb[:], ident_sb[:])
            cdrow_sb = wp.tile([1, C], F32, tag="cdrow_sb")
            nc.scalar.copy(cdrow_sb[:], psum_cdrow[:])
            psum_bc = pp1.tile([128, C], F32, tag="bcast")
            nc.tensor.matmul(psum_bc[:], ones_sb[0:1, :], cdrow_sb[:])
            cdb_sb = wp.tile([128, C], F32, tag="cdb")
            nc.scalar.copy(cdb_sb[:], psum_bc[:])

            # ---- anchor-side prep ----
            xT_sb, xsqT_sb = [], []
            for k in range(KD):
                xsb = wp.tile([128, ROWS], F32, tag=f"xT{k}")
                nc.sync.dma_start(xsb[:], xT[k * 128 : (k + 1) * 128, :])
                xT_sb.append(xsb)
                xsq = wp.tile([128, ROWS], F32, tag=f"xsqT{k}")
                nc.scalar.square(xsq[:], xsb[:])
                xsqT_sb.append(xsq)

            # dcw[:, m*C:(m+1)*C] = dc rows for anchor chunk m
            dcw_sb = wp.tile([128, MCH * C], F32, tag="dcw")

            # ---- main loop: dc2 GEMM per 128-anchor chunk, sqrt into dcw ----
            for m in range(MCH):
                sl = slice(m * 128, (m + 1) * 128)
                psum_dc2 = pp2.tile([128, C], F32, tag="dc2")
                for k in range(KD):
                    nc.tensor.matmul(
                        psum_dc2[:], xsqT_sb[k][:, sl], w2T_sb[k][:],
                        start=(k == 0), stop=False,
                    )
                for k in range(KD):
                    nc.tensor.matmul(
                        psum_dc2[:], xT_sb[k][:, sl], m2T_sb[k][:],
                        start=False, stop=False,
                    )
                nc.tensor.matmul(
                    psum_dc2[:], ones_sb[0:1, :], arow_sb[:],
                    start=False, stop=True,
                )
                nc.scalar.sqrt(dcw_sb[:, m * C : (m + 1) * C], psum_dc2[:])

            # ---- batched mining over all 4 chunks: [128, MCH, C] views ----
            dc3 = dcw_sb[:].rearrange("p (m c) -> p m c", c=C)
            iota3 = iota_sb[:, None, :].broadcast_to([128, MCH, C])
            t3 = t_sb[:, :, None].broadcast_to([128, MCH, C])
            cdb3 = cdb_sb[:, None, :].broadcast_to([128, MCH, C])

            ohb_sb = kp.tile([128, MCH * C], F32, tag="ohb")
            oh3 = ohb_sb[:].rearrange("p (m c) -> p m c", c=C)
            nc.vector.tensor_tensor(oh3, iota3, t3, op=mybir.AluOpType.is_equal)
            nc.vector.tensor_scalar(
                ohb_sb[:], ohb_sb[:], BIG, None, op0=mybir.AluOpType.mult
            )
            apt_sb = kp.tile([128, MCH * C], F32, tag="apt")
            nc.vector.tensor_tensor(
                apt_sb[:].rearrange("p (m c) -> p m c", c=C), dc3, oh3,
                op=mybir.AluOpType.mult,
            )
            apB = kp.tile([128, MCH], F32, tag="apB")
            nc.vector.tensor_reduce(
                apB[:], apt_sb[:].rearrange("p (m c) -> p m c", c=C),
                axis=mybir.AxisListType.X, op=mybir.AluOpType.add,
            )
            cct_sb = kp.tile([128, MCH * C], F32, tag="cct")
            nc.vector.tensor_tensor(
                cct_sb[:].rearrange("p (m c) -> p m c", c=C), cdb3, oh3,
                op=mybir.AluOpType.mult,
            )
            ccB = kp.tile([128, MCH], F32, tag="ccB")
            nc.vector.tensor_reduce(
                ccB[:], cct_sb[:].rearrange("p (m c) -> p m c", c=C),
                axis=mybir.AxisListType.X, op=mybir.AluOpType.add,
            )
            ant_sb = kp.tile([128, MCH * C], F32, tag="ant")
            nc.vector.tensor_tensor(
                ant_sb[:].rearrange("p (m c) -> p m c", c=C), dc3, oh3,
                op=mybir.AluOpType.add,
            )
            anneg = kp.tile([128, MCH], F32, tag="anneg")
            nc.vector.tensor_reduce(
                anneg[:], ant_sb[:].rearrange("p (m c) -> p m c", c=C),
                axis=mybir.AxisListType.X, op=mybir.AluOpType.min, negate=True,
            )
            # margin = relu(cc - an) ; cc = ccB / BIG ; -an = anneg
            cc4 = kp.tile([128, MCH], F32, tag="cc4")
            nc.scalar.mul(cc4[:], ccB[:], 1.0 / BIG)
            mrg_in = kp.tile([128, MCH], F32, tag="mrgin")
            nc.vector.tensor_tensor(
                mrg_in[:], cc4[:], anneg[:], op=mybir.AluOpType.add
            )
            mrg = kp.tile([128, MCH], F32, tag="mrg")
            nc.scalar.activation(
                mrg[:], mrg_in[:], mybir.ActivationFunctionType.Relu
            )
            ap4 = kp.tile([128, MCH], F32, tag="ap4")
            nc.scalar.mul(ap4[:], apB[:], 1.0 / BIG)
            loss4 = kp.tile([128, MCH], F32, tag="loss4")
            nc.vector.tensor_tensor(
                loss4[:], mrg[:], ap4[:], op=mybir.AluOpType.add
            )
            losscol = kp.tile([128, 1], F32, tag="losscol")
            nc.vector.tensor_reduce(
                losscol[:], loss4[:], axis=mybir.AxisListType.X,
                op=mybir.AluOpType.add,
            )

            # ---- partial sum over this core's 512 rows ----
            psum_loss = pp1.tile([1, 1], F32, tag="loss")
            nc.tensor.matmul(psum_loss[:], ones_sb[:, 0:1], losscol[:])
            res_sb = wp.tile([1, 1], F32, tag="res")
            nc.scalar.copy(res_sb[:], psum_loss[:])
            nc.sync.dma_start(out[:], res_sb[:])

    nc.compile()
    return nc


_NC_CACHE: list = []


def _get_nc() -> bass.Bass:
    if not _NC_CACHE:
        _NC_CACHE.append(build_nc())
    return _NC_CACHE[0]


def make_in_maps(inputs, centers, centers_weights, targets):
    x = np.ascontiguousarray(np.asarray(inputs, dtype=np.float32))
    c = np.asarray(centers, dtype=np.float32)
    cw = np.asarray(centers_weights, dtype=np.float32)
    t = np.asarray(targets).astype(np.int64)

    xT = np.ascontiguousarray(x.T)                      # [D, B]
    cT = np.ascontiguousarray(c.T)                      # [D, C]
    cwT = np.ascontiguousarray(cw.T)                    # [D, C]
    pen = np.zeros((1, C), dtype=np.float32)
    present = np.zeros(C, dtype=bool)
    present[np.unique(t)] = True
    pen[0, ~present] = PEN
    iota = np.tile(np.arange(C, dtype=np.float32), (128, 1))
    ident = np.eye(C, dtype=np.float32)
    ones = np.ones((128, 128), dtype=np.float32)

    in_maps = []
    for i in range(NCORES):
        rows = slice(i * ROWS, (i + 1) * ROWS)
        in_maps.append({
            "xT": np.ascontiguousarray(xT[:, rows]),
            "cT": cT,
            "cwT": cwT,
            "tgt": t[rows].astype(np.float32).reshape(MCH, 128),
            "pen": pen,
            "iota": iota,
            "ident": ident,
            "ones": ones,
        })
    return in_maps


def kernel(inputs, centers, centers_weights, targets, epoch_number=None,
           **_ignored):
    nc = _get_nc()
    in_maps = make_in_maps(inputs, centers, centers_weights, targets)
    res = run_bass_kernel_spmd(nc, in_maps, core_ids=list(range(NCORES)))
    total = sum(float(r["out"][0, 0]) for r in res.results)
    return np.float32(total / B)
